# revision 1
# baseline (speedup 1.0000x reference)
"""Trainium2 Bass kernel for EnhancedLinkPredictor (GNN common-neighbor link prediction).

Math (per prediction edge e=(s,d)):
  shared_ddi = adj_ddi[s] & adj_ddi[d]          (drug-drug, N_D=8192)
  cn_ddi     = (shared_ddi @ z_drug)  / max(|shared_ddi|, 1)
  shared_dp  = adj_dp[s]  & adj_dp[d]           (drug-protein, N_P=4096)
  cn_prot    = (shared_dp @ z_protein) / max(|shared_dp|, 1)
  pair  = [z_drug[s], z_drug[d], cn_ddi, cn_prot]   (256)
  out   = sigmoid(relu(pair @ W1 + b1) @ W2 + b2)

Device strategy (8 cores, data-parallel over the 16384 pred edges, 2048/core):
  - Adjacency is nibble-packed host-side (2 entries/byte, code 0x7), compacted
    per core to the <=4096 rows that core touches. The ddi row additionally
    carries the drug's z row as 128 bf16 (256B), so one gather per edge
    endpoint fetches adjacency + embedding.
  - dma_gather(transpose=True, uint16 view) pulls whole rows (s and d idx
    lists merged per call) and lands them transposed: partition p of 256B
    chunk c holds packed bytes for drugs 512c+4p..4p+3. One endpoint visit
    total per relation (~8.4ns/idx GPSIMD descriptor-gen is the constraint).
  - AND runs on packed data (uint32 bitwise_and, half the words), then two
    fused tensor_scalar ops expand nibbles to exact fp8 {0, 0x38=1.0}:
    parity0 = (x & 0x0707..) << 3, parity1 = (x & 0x7070..) >> 1.
  - Matmuls run in fp8 DoubleRow (partition p sub-row i contracts
    k = 512c + 4p + m + 2i) with Z split into two e4m3 limbs for ~bf16
    accuracy, f32 PSUM; Z column 64 is ones so intersection counts fall out
    of the same matmul. Host packs Z in exactly this permutation.
  - Everything downstream is computed transposed ([dim, edge]); the MLP
    consumes pair^T directly.
"""

import numpy as np
import ml_dtypes
from contextlib import ExitStack

import concourse.bass as bass
import concourse.bacc as bacc
import concourse.mybir as mybir
import concourse.tile as tile

N_D, N_P = 8192, 4096
D_DIM, HID = 64, 128
E_PRED = 16384
N_CORES = 8
E_LOC = E_PRED // N_CORES          # 2048 edges per core
U_PAD = 4096                       # compacted adjacency row count (>= unique refs)

MCOL = 80                          # padded Z columns (64 dims + 1 ones + 15 pad)
KBLK = 512                         # k per packed 256B chunk

C_DDI = N_D // KBLK                # 16 packed chunks per ddi row
C_DDIZ = C_DDI + 1                 # + z chunk
C_DP = N_P // KBLK                 # 8 packed chunks per dp row
G_DDI = 128                        # edges per ddi gather call (s+d merged: 256 idxs)
G_DP = 256                         # edges per dp gather call (s+d merged: 512 idxs)
N_G = E_LOC // G_DDI               # 16 ddi calls
N_H = E_LOC // G_DP                # 8 dp calls
N_ET = E_LOC // 512                # 4 supertiles of 512 edges
IDX_COLS = N_G * (2 * G_DDI // 16) + N_H * (2 * G_DP // 16)  # 512
USE_DOUBLEROW = True

FP8 = ml_dtypes.float8_e4m3
BF16 = ml_dtypes.bfloat16

MASK_LO = 0x07070707
MASK_HI = 0x70707070


def _fp8_limbs(x: np.ndarray):
    hi = x.astype(FP8)
    lo = (x - hi.astype(np.float32)).astype(FP8)
    return hi.view(np.uint8), lo.view(np.uint8)


def _pack_z_doublerow(z: np.ndarray):
    """z [K, 64] f32 -> [128, n_groups*2*MCOL] uint8 fp8 DoubleRow lhsT blocks.
    Group g = limb*(2*n512) + c*2 + m holds rows k = 512c + 4p + m + 2i at
    (partition p, sub-row i). Column 64 is ones (hi limb)."""
    K = z.shape[0]
    zext = np.zeros((K, MCOL), dtype=np.float32)
    zext[:, :D_DIM] = z
    zext[:, D_DIM] = 1.0
    hi, lo = _fp8_limbs(zext)
    n512 = K // KBLK
    p = np.arange(128)[:, None]
    i = np.arange(2)[None, :]
    out = np.empty((2 * 2 * n512, 128, 2, MCOL), dtype=np.uint8)
    for li, limb in enumerate((hi, lo)):
        for c in range(n512):
            for m in range(2):
                ks = 512 * c + 4 * p + m + 2 * i  # [128, 2]
                out[li * 2 * n512 + c * 2 + m] = limb[ks]
    return np.ascontiguousarray(out.transpose(1, 0, 2, 3).reshape(128, -1))


def _wrap_idxs(idx: np.ndarray):
    """[n] int -> [128, n/16] int16 wrapped (j -> [j%16, j//16]) + 8x replicated."""
    n = idx.shape[0]
    w = np.zeros((16, n // 16), dtype=np.int16)
    w[np.arange(n) % 16, np.arange(n) // 16] = idx.astype(np.int16)
    return np.tile(w, (8, 1))


def build_body(tc, t):
    """Emit the per-core program. t: dict name -> AP of DRAM tensors."""
    nc = tc.nc
    dt = mybir.dt
    with ExitStack() as ctx:
        const = ctx.enter_context(tc.tile_pool(name="const", bufs=1))
        gpool = ctx.enter_context(tc.tile_pool(name="gath", bufs=6))
        kpool = ctx.enter_context(tc.tile_pool(name="pckm", bufs=3))
        mpool = ctx.enter_context(tc.tile_pool(name="mask", bufs=2))
        tails = ctx.enter_context(tc.tile_pool(name="tails", bufs=2))
        pairp = ctx.enter_context(tc.tile_pool(name="pair", bufs=1))
        psum = ctx.enter_context(tc.tile_pool(name="ps", bufs=8, space="PSUM"))

        # idx on the sync queue (gathers depend on it); bulk constants on the
        # scalar HWDGE queue so they don't delay the first gather.
        idxt = const.tile([128, IDX_COLS], dt.int16)
        nc.sync.dma_start(idxt[:], t["IDX"][:, :])

        zd = const.tile([128, 4 * C_DDI * 2 * MCOL], dt.uint8)
        nc.scalar.dma_start(zd[:], t["ZD"][:, :])
        zp = const.tile([128, 4 * C_DP * 2 * MCOL], dt.uint8)
        nc.scalar.dma_start(zp[:], t["ZP"][:, :])
        w1t = const.tile([64, 4 * HID], dt.uint16)
        nc.scalar.dma_start(w1t[:], t["W1"][:, :])
        w2t = const.tile([128, 1], dt.uint16)
        nc.scalar.dma_start(w2t[:], t["W2"][:, :])
        b1t = const.tile([128, 1], dt.float32)
        nc.scalar.dma_start(b1t[:], t["B1"][:, :])
        b2t = const.tile([1, 1], dt.float32)
        nc.scalar.dma_start(b2t[:], t["B2"][:, :])
        ones = const.tile([1, 64], dt.float32)
        nc.vector.memset(ones[:], 1.0)

        zsrc = pairp.tile([128, E_LOC], dt.uint16)
        zdst = pairp.tile([128, E_LOC], dt.uint16)

        cn_ps = {}
        for et in range(N_ET):
            cn_ps[("ddi", et)] = psum.tile(
                [MCOL, 512], dt.float32, tag="ps", name=f"cn_ddi{et}"
            )
            cn_ps[("dp", et)] = psum.tile(
                [MCOL, 512], dt.float32, tag="ps", name=f"cn_dp{et}"
            )

        def expand(mkp, msk, n_chunks, ecols, e0):
            """Nibble-expand packed masked [128, n_chunks, ecols/2 u32] into
            fp8 parity planes of msk [128, n_chunks, 2, 512 u16]."""
            w = ecols // 2  # u32 words per chunk
            i32 = mkp[:].bitcast(dt.uint32).rearrange("p (c w) -> p c w", c=n_chunks)
            o32 = msk[:].bitcast(dt.uint32).rearrange(
                "p (c m w) -> p c m w", c=n_chunks, m=2
            )
            nc.vector.tensor_scalar(
                o32[:, :, 0, e0 // 2:e0 // 2 + w],
                i32[:, :, :],
                MASK_LO,
                3,
                mybir.AluOpType.bitwise_and,
                mybir.AluOpType.logical_shift_left,
            )
            nc.vector.tensor_scalar(
                o32[:, :, 1, e0 // 2:e0 // 2 + w],
                i32[:, :, :],
                MASK_HI,
                1,
                mybir.AluOpType.bitwise_and,
                mybir.AluOpType.logical_shift_right,
            )

        def matmuls(rel, et, msk, zt, n_chunks):
            m8 = msk[:].bitcast(dt.float8e4).rearrange(
                "p (c m i two) -> p c m two i", c=n_chunks, m=2, two=2
            )
            zt8 = zt[:].bitcast(dt.float8e4).rearrange(
                "p (g two m) -> p g two m", g=4 * n_chunks, two=2
            )
            n_mm = 0
            for c in range(n_chunks):
                for m in range(2):
                    for limb in range(2):
                        g = limb * 2 * n_chunks + c * 2 + m
                        n_mm += 1
                        first = n_mm == 1
                        last = n_mm == 4 * n_chunks
                        if USE_DOUBLEROW:
                            nc.tensor.matmul(
                                cn_ps[(rel, et)][:],
                                zt8[:, g, :, :],
                                m8[:, c, m, :, :],
                                start=first,
                                stop=last,
                                perf_mode=mybir.MatmulPerfMode.DoubleRow,
                            )
                        else:
                            for par in range(2):
                                nc.tensor.matmul(
                                    cn_ps[(rel, et)][:],
                                    zt8[:, g, par, :],
                                    m8[:, c, m, par, :],
                                    start=first and par == 0,
                                    stop=last and par == 1,
                                )

        cn_sb = {
            "ddi": pairp.tile([64, E_LOC], dt.bfloat16, tag="cnddi", name="cnddi"),
            "dp": pairp.tile([64, E_LOC], dt.bfloat16, tag="cndp", name="cndp"),
        }
        dp_base = N_G * (2 * G_DDI // 16)

        def ddi_supertile(et):
            msk = mpool.tile(
                [128, C_DDI * 2 * 512], dt.uint16, tag="mka", name=f"mka{et}"
            )
            for sub in range(4):
                g = 4 * et + sub
                gt = gpool.tile([128, C_DDIZ * 2 * G_DDI], dt.uint16, tag="gt")
                ncols = 2 * G_DDI // 16
                nc.gpsimd.dma_gather(
                    out_ap=gt[:].rearrange("p (c i) -> p c i", c=C_DDIZ),
                    in_ap=t["A1"][:, :],
                    idxs_ap=idxt[:, g * ncols:(g + 1) * ncols],
                    num_idxs=2 * G_DDI,
                    num_idxs_reg=2 * G_DDI,
                    elem_size=C_DDIZ * 128,
                    elem_step=C_DDIZ * 128,
                    transpose=True,
                    single_packet=False,
                )
                # z chunk -> pair^T rows (s first half, d second half)
                gv = gt[:].rearrange("p (c i) -> p c i", c=C_DDIZ)
                nc.vector.tensor_copy(
                    zsrc[:, G_DDI * g:G_DDI * (g + 1)], gv[:, C_DDI, 0:G_DDI]
                )
                nc.vector.tensor_copy(
                    zdst[:, G_DDI * g:G_DDI * (g + 1)], gv[:, C_DDI, G_DDI:2 * G_DDI]
                )
                g32 = gt[:].bitcast(dt.uint32).rearrange("p (c i) -> p c i", c=C_DDIZ)
                mkp = kpool.tile([128, C_DDI * G_DDI // 2], dt.uint32, tag="mkp")
                kv = mkp[:].rearrange("p (c w) -> p c w", c=C_DDI)
                hw = G_DDI // 2
                nc.vector.tensor_tensor(
                    kv[:, :, :],
                    g32[:, :C_DDI, 0:hw],
                    g32[:, :C_DDI, hw:2 * hw],
                    mybir.AluOpType.bitwise_and,
                )
                expand(mkp, msk, C_DDI, G_DDI, sub * G_DDI)
            matmuls("ddi", et, msk, zd, C_DDI)

        def dp_supertile(et):
            msk = mpool.tile(
                [128, C_DP * 2 * 512], dt.uint16, tag="mkb", name=f"mkb{et}", bufs=1
            )
            for sub in range(2):
                h = 2 * et + sub
                gt = gpool.tile([128, C_DP * 2 * G_DP], dt.uint16, tag="gt")
                ncols = 2 * G_DP // 16
                nc.gpsimd.dma_gather(
                    out_ap=gt[:].rearrange("p (c i) -> p c i", c=C_DP),
                    in_ap=t["A2"][:, :],
                    idxs_ap=idxt[:, dp_base + h * ncols:dp_base + (h + 1) * ncols],
                    num_idxs=2 * G_DP,
                    num_idxs_reg=2 * G_DP,
                    elem_size=C_DP * 128,
                    elem_step=C_DP * 128,
                    transpose=True,
                    single_packet=False,
                )
                g32 = gt[:].bitcast(dt.uint32).rearrange("p (c i) -> p c i", c=C_DP)
                mkp = kpool.tile([128, C_DP * G_DP // 2], dt.uint32, tag="mkp")
                kv = mkp[:].rearrange("p (c w) -> p c w", c=C_DP)
                hw = G_DP // 2
                nc.vector.tensor_tensor(
                    kv[:, :, :],
                    g32[:, :, 0:hw],
                    g32[:, :, hw:2 * hw],
                    mybir.AluOpType.bitwise_and,
                )
                expand(mkp, msk, C_DP, G_DP, sub * G_DP)
            matmuls("dp", et, msk, zp, C_DP)

        def normalize(rel, et):
            ps = cn_ps[(rel, et)]
            raw = tails.tile([66, 512], dt.float32, tag="raw")
            nc.scalar.copy(raw[:], ps[0:66, :])
            rec = tails.tile([1, 512], dt.float32, tag="rec")
            nc.vector.tensor_scalar_max(rec[:], raw[64:65, :], 1.0)
            nc.vector.reciprocal(rec[:], rec[:])
            bc = psum.tile([64, 512], dt.float32, tag="ps")
            nc.tensor.matmul(bc[:], ones[:], rec[:], start=True, stop=True)
            nc.vector.tensor_tensor(
                cn_sb[rel][:, 512 * et:512 * (et + 1)],
                raw[0:64, :],
                bc[:],
                mybir.AluOpType.mult,
            )

        def mlp(et):
            hps = psum.tile([HID, 512], dt.float32, tag="ps")
            rhs_chunks = (
                zsrc[:].bitcast(dt.bfloat16)[0:64, 512 * et:512 * (et + 1)],
                zdst[:].bitcast(dt.bfloat16)[0:64, 512 * et:512 * (et + 1)],
                cn_sb["ddi"][:, 512 * et:512 * (et + 1)],
                cn_sb["dp"][:, 512 * et:512 * (et + 1)],
            )
            for j, rhs in enumerate(rhs_chunks):
                nc.tensor.matmul(
                    hps[:],
                    w1t[:].bitcast(dt.bfloat16)[:, HID * j:HID * (j + 1)],
                    rhs,
                    start=(j == 0),
                    stop=(j == 3),
                )
            hsb = tails.tile([HID, 512], dt.bfloat16, tag="h")
            nc.scalar.activation(
                hsb[:], hps[:], mybir.ActivationFunctionType.Relu, bias=b1t[:, 0:1]
            )
            lps = psum.tile([1, 512], dt.float32, tag="ps")
            nc.tensor.matmul(
                lps[:], w2t[:].bitcast(dt.bfloat16), hsb[:], start=True, stop=True
            )
            osb = tails.tile([1, 512], dt.float32, tag="osb")
            nc.scalar.activation(
                osb[:],
                lps[:],
                mybir.ActivationFunctionType.Sigmoid,
                bias=b2t[:, 0:1],
            )
            nc.sync.dma_start(t["OUT"][:, 512 * et:512 * (et + 1)], osb[:])

        for et in range(N_ET):
            ddi_supertile(et)
            dp_supertile(et)
            normalize("ddi", et)
            normalize("dp", et)
            mlp(et)


def build_program():
    nc = bacc.Bacc("TRN2", target_bir_lowering=False)
    dt = mybir.dt
    t = {
        "A1": nc.dram_tensor(
            "A1", [U_PAD, C_DDIZ * 128], dt.uint16, kind="ExternalInput"
        ).ap(),
        "A2": nc.dram_tensor(
            "A2", [U_PAD, C_DP * 128], dt.uint16, kind="ExternalInput"
        ).ap(),
        "IDX": nc.dram_tensor("IDX", [128, IDX_COLS], dt.int16, kind="ExternalInput").ap(),
        "ZD": nc.dram_tensor(
            "ZD", [128, 4 * C_DDI * 2 * MCOL], dt.uint8, kind="ExternalInput"
        ).ap(),
        "ZP": nc.dram_tensor(
            "ZP", [128, 4 * C_DP * 2 * MCOL], dt.uint8, kind="ExternalInput"
        ).ap(),
        "W1": nc.dram_tensor("W1", [64, 4 * HID], dt.uint16, kind="ExternalInput").ap(),
        "B1": nc.dram_tensor("B1", [HID, 1], dt.float32, kind="ExternalInput").ap(),
        "W2": nc.dram_tensor("W2", [HID, 1], dt.uint16, kind="ExternalInput").ap(),
        "B2": nc.dram_tensor("B2", [1, 1], dt.float32, kind="ExternalInput").ap(),
        "OUT": nc.dram_tensor("OUT", [1, E_LOC], dt.float32, kind="ExternalOutput").ap(),
    }
    with tile.TileContext(nc) as tc:
        build_body(tc, t)
    nc.compile()
    return nc


def host_prep(z_drug, z_protein, ddi_ei, dp_ei, pred_ei, W1, b1, W2, b2):
    """Build the 8 per-core input maps (all numpy, no device work)."""
    z_drug = np.asarray(z_drug, np.float32)
    z_protein = np.asarray(z_protein, np.float32)
    ddi_ei = np.asarray(ddi_ei, np.int64)
    dp_ei = np.asarray(dp_ei, np.int64)
    pred_ei = np.asarray(pred_ei, np.int64)

    A_ddi = np.zeros((N_D, N_D), dtype=np.uint8)
    A_ddi[ddi_ei[0], ddi_ei[1]] = 1
    A_ddi[ddi_ei[1], ddi_ei[0]] = 1
    A_dp = np.zeros((N_D, N_P), dtype=np.uint8)
    A_dp[dp_ei[0], dp_ei[1]] = 1
    # nibble pack: byte B = drug 2B (low nibble 0x7) | drug 2B+1 (high 0x70)
    A_ddi_nib = (A_ddi[:, 0::2] * 0x07) | (A_ddi[:, 1::2] * 0x70)
    A_dp_nib = (A_dp[:, 0::2] * 0x07) | (A_dp[:, 1::2] * 0x70)

    zb_full = np.zeros((N_D, 128), dtype=np.float32)
    zb_full[:, :D_DIM] = z_drug
    zb_bytes = zb_full.astype(BF16).view(np.uint8)  # [N_D, 256]

    ZD = _pack_z_doublerow(z_drug)
    ZP = _pack_z_doublerow(z_protein)
    W1p = np.ascontiguousarray(
        np.asarray(W1, np.float32)
        .reshape(4, 64, HID)
        .astype(BF16)
        .view(np.uint16)
        .transpose(1, 0, 2)
        .reshape(64, 4 * HID)
    )
    B1 = np.asarray(b1, np.float32).reshape(HID, 1)
    W2p = np.asarray(W2, np.float32).reshape(HID, 1).astype(BF16).view(np.uint16)
    B2 = np.asarray(b2, np.float32).reshape(1, 1)

    in_maps = []
    for c in range(N_CORES):
        s = pred_ei[0, c * E_LOC:(c + 1) * E_LOC]
        d = pred_ei[1, c * E_LOC:(c + 1) * E_LOC]
        rows = np.unique(np.concatenate([s, d]))
        nu = rows.shape[0]
        assert nu <= U_PAD
        remap_s = np.searchsorted(rows, s).astype(np.int16)
        remap_d = np.searchsorted(rows, d).astype(np.int16)
        A1 = np.zeros((U_PAD, C_DDIZ * 256), dtype=np.uint8)
        A1[:nu, : N_D // 2] = A_ddi_nib[rows]
        A1[:nu, N_D // 2:] = zb_bytes[rows]
        A2 = np.zeros((U_PAD, N_P // 2), dtype=np.uint8)
        A2[:nu] = A_dp_nib[rows]

        cols = []
        for g in range(N_G):
            cols.append(
                _wrap_idxs(
                    np.concatenate(
                        [remap_s[G_DDI * g:G_DDI * (g + 1)],
                         remap_d[G_DDI * g:G_DDI * (g + 1)]]
                    )
                )
            )
        for h in range(N_H):
            cols.append(
                _wrap_idxs(
                    np.concatenate(
                        [remap_s[G_DP * h:G_DP * (h + 1)],
                         remap_d[G_DP * h:G_DP * (h + 1)]]
                    )
                )
            )
        idx = np.concatenate(cols, axis=1)
        assert idx.shape == (128, IDX_COLS)

        in_maps.append(
            {
                "A1": A1.view(np.uint16),
                "A2": A2.view(np.uint16),
                "IDX": idx,
                "ZD": ZD,
                "ZP": ZP,
                "W1": W1p,
                "B1": B1,
                "W2": W2p,
                "B2": B2,
            }
        )
    return in_maps


def kernel(z_drug, z_protein, ddi_ei, dp_ei, pred_ei, W1, b1, W2, b2, _profile=None):
    from concourse.bass_utils import run_bass_kernel_spmd

    in_maps = host_prep(z_drug, z_protein, ddi_ei, dp_ei, pred_ei, W1, b1, W2, b2)
    nc = build_program()
    res = run_bass_kernel_spmd(
        nc,
        in_maps,
        core_ids=list(range(N_CORES)),
        **({} if _profile is None else _profile),
    )
    if _profile is not None:
        kernel.last_results = res
    out = np.concatenate([r["OUT"].reshape(-1) for r in res.results])
    return out.astype(np.float32)



# revision 15
# speedup vs baseline: 1.3172x; 1.3172x over previous
"""Trainium2 Bass kernel for EnhancedLinkPredictor (GNN common-neighbor link prediction).

Math (per prediction edge e=(s,d)):
  shared_ddi = adj_ddi[s] & adj_ddi[d]          (drug-drug, N_D=8192)
  cn_ddi     = (shared_ddi @ z_drug)  / max(|shared_ddi|, 1)
  shared_dp  = adj_dp[s]  & adj_dp[d]           (drug-protein, N_P=4096)
  cn_prot    = (shared_dp @ z_protein) / max(|shared_dp|, 1)
  pair  = [z_drug[s], z_drug[d], cn_ddi, cn_prot]   (256)
  out   = sigmoid(relu(pair @ W1 + b1) @ W2 + b2)

Device strategy (8 cores, data-parallel over the 16384 pred edges, 2048/core):
  - Adjacency is BIT-packed host-side (u16 word w bit b = neighbor 16w+b),
    compacted per core to the <=4096 rows that core touches. One merged row =
    [ddi bits 1024B | z_drug bf16 256B | dp bits 512B] = 1792B, so ONE
    transposed dma_gather per edge-endpoint fetches everything (7 RX
    descriptors/idx instead of 25 in the nibble scheme).
  - Per 512-edge supertile one gather call of 1024 idxs (s then d). After
    transpose, partition p of chunk c holds the u16 covering neighbors
    2048c + 16p .. +15 of one endpoint.
  - AND runs on packed u32 words (GpSimd). Bit->fp8 expansion: plane m
    selects bits m / m+8 of each u16 ((x & (0x0101<<m)) shifted so the bit
    lands at position 3 of its byte) = fp8 bytes {0, 0x08=2^-6}. The uniform
    2^-6 scale cancels in cn = S/max(cnt,2^-6). Planes are split between
    DVE and GpSimd (tunable).
  - Matmuls are fp8 DoubleRow: partition p sub-row i contracts
    k = 2048c + 16p + 8i + m. Weights pack BOTH z limbs in one 128-col block
    [z_hi 64 | ones | z_lo 63] (lo of dim 63 dropped), so each mask block is
    streamed ONCE; counts fall out of col 64. hi+lo recombined on DVE.
  - MLP consumes pair^T directly; z rows come straight from the gather tile.
"""

import numpy as np
import ml_dtypes
from contextlib import ExitStack

import concourse.bass as bass
import concourse.bacc as bacc
import concourse.mybir as mybir
import concourse.tile as tile

N_D, N_P = 8192, 4096
D_DIM, HID = 64, 128
E_PRED = 16384
N_CORES = 8
E_LOC = E_PRED // N_CORES          # 2048 edges per core
U_PAD = 4096                       # compacted adjacency row count (>= unique refs)

KBLK = 2048                        # neighbors per 128-u16 chunk
C_DDI = N_D // KBLK                # 4 ddi chunks
C_DP = N_P // KBLK                 # 2 dp chunks
C_ROW = C_DDI + 1 + C_DP           # 7 chunks per merged row (u16 x128 each)
ROW_U16 = C_ROW * 128              # 896 u16 = 1792B per row
N_ET = 4                           # supertiles of 512 edges
G_IDX = 1024                       # idxs per gather call (512 s + 512 d)
IDX_COLS = N_ET * (G_IDX // 16)    # 256

# which dp expansion planes run on GpSimd (rest on DVE); ddi planes on DVE
DP_PLANES_POOL = ()

FP8 = ml_dtypes.float8_e4m3
BF16 = ml_dtypes.bfloat16


def _fp8_limbs(x: np.ndarray):
    hi = x.astype(FP8)
    lo = (x - hi.astype(np.float32)).astype(FP8)
    return hi.view(np.uint8), lo.view(np.uint8)


# cn rows come out permuted: partition p holds dim PERM[p]. Chosen so the
# hi+lo combine and count extraction only touch aligned partition blocks:
# col p = hi of dim PERM[p]; col 64 = ones; col 64+p (p>0) = lo of dim PERM[p].
# Row 0 of (hi+lo) picks up the count (corrected later); dim 63 loses its lo.
PERM = np.array([63] + list(range(63)))


def _pack_z_doublerow(z: np.ndarray):
    """z [K, 64] f32 -> [128, n_groups*2*128] uint8 fp8 DoubleRow lhsT blocks.
    Cols = [hi[PERM] 64 | ones @64 | lo[PERM[1:]] 63]. Group g = c*8+m holds
    rows k = 2048c + 16p + 8i + m at (partition p, sub-row i)."""
    K = z.shape[0]
    hi, lo = _fp8_limbs(z)
    zc = np.zeros((K, 128), dtype=np.uint8)
    zc[:, :64] = hi[:, PERM]
    zc[:, 64] = np.float32(1.0).astype(FP8).view(np.uint8)
    zc[:, 65:128] = lo[:, PERM[1:]]
    nch = K // KBLK
    p = np.arange(128)[:, None]
    i = np.arange(2)[None, :]
    out = np.empty((nch * 8, 128, 2, 128), dtype=np.uint8)
    for c in range(nch):
        for m in range(8):
            ks = 2048 * c + 16 * p + 8 * i + m  # [128, 2]
            out[c * 8 + m] = zc[ks]
    return np.ascontiguousarray(out.transpose(1, 0, 2, 3).reshape(128, -1))


def _wrap_idxs(idx: np.ndarray):
    """[n] int -> [128, n/16] int16 wrapped (j -> [j%16, j//16]) + 8x replicated."""
    n = idx.shape[0]
    w = np.zeros((16, n // 16), dtype=np.int16)
    w[np.arange(n) % 16, np.arange(n) // 16] = idx.astype(np.int16)
    return np.tile(w, (8, 1))


def build_body(tc, t):
    nc = tc.nc
    dt = mybir.dt
    AL = mybir.AluOpType
    with ExitStack() as ctx:
        const = ctx.enter_context(tc.tile_pool(name="const", bufs=1))
        gpool = ctx.enter_context(tc.tile_pool(name="gath", bufs=3))
        apool = ctx.enter_context(tc.tile_pool(name="andp", bufs=2))
        mpool = ctx.enter_context(tc.tile_pool(name="mask", bufs=2))
        tails = ctx.enter_context(tc.tile_pool(name="tails", bufs=3))
        pairp = ctx.enter_context(tc.tile_pool(name="pair", bufs=1))
        psum = ctx.enter_context(tc.tile_pool(name="ps", bufs=8, space="PSUM"))

        idxt = const.tile([128, IDX_COLS], dt.int16)
        nc.sync.dma_start(idxt[:], t["IDX"][:, :])

        zd = const.tile([128, C_DDI * 8 * 2 * 128], dt.uint8)
        nc.scalar.dma_start(zd[:], t["ZD"][:, :])
        zp = const.tile([128, C_DP * 8 * 2 * 128], dt.uint8)
        nc.scalar.dma_start(zp[:], t["ZP"][:, :])
        w1t = const.tile([64, 4 * HID], dt.uint16)
        nc.scalar.dma_start(w1t[:], t["W1"][:, :])
        w2t = const.tile([128, 1], dt.uint16)
        nc.scalar.dma_start(w2t[:], t["W2"][:, :])
        b1t = const.tile([128, 1], dt.float32)
        nc.scalar.dma_start(b1t[:], t["B1"][:, :])
        b2t = const.tile([1, 1], dt.float32)
        nc.scalar.dma_start(b2t[:], t["B2"][:, :])
        ones = const.tile([1, 64], dt.bfloat16)
        nc.vector.memset(ones[:], 1.0)

        zt8d = zd[:].bitcast(dt.float8e4).rearrange(
            "p (g two m) -> p g two m", g=C_DDI * 8, two=2
        )
        zt8p = zp[:].bitcast(dt.float8e4).rearrange(
            "p (g two m) -> p g two m", g=C_DP * 8, two=2
        )

        cn_sb = {
            "ddi": pairp.tile([64, E_LOC], dt.bfloat16, tag="cnddi", name="cnddi"),
            "dp": pairp.tile([64, E_LOC], dt.bfloat16, tag="cndp", name="cndp"),
        }

        def expand(eng, out16, in16, m):
            """out = (in & (0x0101<<m)) shifted so bit m lands at bit 3."""
            pm = (0x0101 << m) & 0xFFFF
            if m < 3:
                eng.tensor_scalar(out16, in16, pm, 3 - m, AL.bitwise_and,
                                  AL.logical_shift_left)
            elif m == 3:
                eng.tensor_scalar(out16, in16, pm, None, AL.bitwise_and)
            else:
                eng.tensor_scalar(out16, in16, pm, m - 3, AL.bitwise_and,
                                  AL.logical_shift_right)

        def supertile(et):
            gt = gpool.tile([128, C_ROW * G_IDX], dt.uint16, tag="gt")
            gv = gt[:].rearrange("p (c i) -> p c i", c=C_ROW)
            nc.gpsimd.dma_gather(
                out_ap=gv,
                in_ap=t["A1"][:, :],
                idxs_ap=idxt[:, et * (G_IDX // 16):(et + 1) * (G_IDX // 16)],
                num_idxs=G_IDX,
                num_idxs_reg=G_IDX,
                elem_size=ROW_U16,
                elem_step=ROW_U16,
                transpose=True,
                single_packet=False,
            )
            g16 = gt[:].rearrange("p (c w) -> p c w", c=C_ROW)
            # intersection on packed bits (s half & d half), per relation.
            # u16 views: u32 bitwise is DVE-only and u16 gets the 2x DVE mode.
            andd = apool.tile([128, C_DDI * 512], dt.uint16, tag="ad")
            adv = andd[:].rearrange("p (c w) -> p c w", c=C_DDI)
            nc.vector.tensor_tensor(
                adv[:, :, :], g16[:, 0:C_DDI, 0:512], g16[:, 0:C_DDI, 512:1024],
                AL.bitwise_and,
            )
            andp = apool.tile([128, C_DP * 512], dt.uint16, tag="aq")
            apv = andp[:].rearrange("p (c w) -> p c w", c=C_DP)
            nc.vector.tensor_tensor(
                apv[:, :, :], g16[:, C_DDI + 1:C_ROW, 0:512],
                g16[:, C_DDI + 1:C_ROW, 512:1024], AL.bitwise_and,
            )
            # bit -> fp8 {0, 2^-6} parity planes
            mka = mpool.tile([128, C_DDI * 8 * 512], dt.uint16, tag="mka")
            ma = mka[:].rearrange("p (c m w) -> p c m w", c=C_DDI, m=8)
            ad16 = adv
            for m in range(8):
                expand(nc.vector, ma[:, :, m, :], ad16[:, :, :], m)
            mkb = mpool.tile([128, C_DP * 8 * 512], dt.uint16, tag="mkb")
            mb = mkb[:].rearrange("p (c m w) -> p c m w", c=C_DP, m=8)
            ap16 = apv
            for m in range(8):
                eng = nc.gpsimd if m in DP_PLANES_POOL else nc.vector
                expand(eng, mb[:, :, m, :], ap16[:, :, :], m)

            # mask @ [z_hi | ones | z_lo] fp8 DoubleRow, one stream per (c, m)
            m8a = mka[:].bitcast(dt.float8e4).rearrange(
                "p (c m e two) -> p c m two e", c=C_DDI, m=8, two=2
            )
            m8b = mkb[:].bitcast(dt.float8e4).rearrange(
                "p (c m e two) -> p c m two e", c=C_DP, m=8, two=2
            )
            ps = {}
            for rel, nch, zt8, m8 in (("ddi", C_DDI, zt8d, m8a),
                                      ("dp", C_DP, zt8p, m8b)):
                pst = psum.tile([128, 512], dt.float32, tag="ps", name=f"cn{rel}{et}")
                ps[rel] = pst
                n_mm = 0
                for c in range(nch):
                    for m in range(8):
                        n_mm += 1
                        nc.tensor.matmul(
                            pst[:],
                            zt8[:, c * 8 + m, :, :],
                            m8[:, c, m, :, :],
                            start=(n_mm == 1),
                            stop=(n_mm == 8 * nch),
                            perf_mode=mybir.MatmulPerfMode.DoubleRow,
                        )

            # normalize: cn = (hi + lo) / max(cnt, 2^-6)   (2^-6 mask scale).
            # tt row 0 = hi[dim63] + cnt; since cnt*rec = (cnt>0), subtract
            # that indicator from row 0 after the scale.
            for rel in ("ddi", "dp"):
                pst = ps[rel]
                rawA = tails.tile([64, 512], dt.float32, tag="rawA")
                nc.scalar.copy(rawA[:], pst[0:64, :])
                rawB = tails.tile([64, 512], dt.float32, tag="rawB")
                nc.scalar.copy(rawB[:], pst[64:128, :])
                recf = tails.tile([1, 512], dt.float32, tag="rec")
                nc.vector.tensor_scalar_max(recf[:], rawB[0:1, :], 0.015625)
                nc.vector.reciprocal(recf[:], recf[:])
                recb = tails.tile([1, 512], dt.bfloat16, tag="recb")
                nc.scalar.copy(recb[:], recf[:])
                bc = psum.tile([64, 512], dt.float32, tag="ps", name=f"bc{rel}{et}")
                nc.tensor.matmul(bc[:], ones[:], recb[:], start=True, stop=True)
                tt = tails.tile([64, 512], dt.float32, tag="tt")
                nc.vector.tensor_tensor(tt[:], rawA[:], rawB[:], AL.add)
                nc.vector.tensor_tensor(
                    tt[0:1, :], tt[0:1, :], rawB[0:1, :], AL.subtract
                )
                cn = cn_sb[rel][:, 512 * et:512 * (et + 1)]
                nc.vector.tensor_tensor(cn, tt[:], bc[:], AL.mult)

            # MLP
            gt16 = gt[:].bitcast(dt.bfloat16).rearrange("p (c i) -> p c i", c=C_ROW)
            hps = psum.tile([HID, 512], dt.float32, tag="ps", name=f"h{et}")
            w1b = w1t[:].bitcast(dt.bfloat16)
            rhs_chunks = (
                gt16[0:64, C_DDI, 0:512],
                gt16[0:64, C_DDI, 512:1024],
                cn_sb["ddi"][:, 512 * et:512 * (et + 1)],
                cn_sb["dp"][:, 512 * et:512 * (et + 1)],
            )
            for j, rhs in enumerate(rhs_chunks):
                nc.tensor.matmul(
                    hps[:], w1b[:, HID * j:HID * (j + 1)], rhs,
                    start=(j == 0), stop=(j == 3),
                )
            hsb = tails.tile([HID, 512], dt.bfloat16, tag="h")
            nc.scalar.activation(
                hsb[:], hps[:], mybir.ActivationFunctionType.Relu, bias=b1t[:, 0:1]
            )
            lps = psum.tile([1, 512], dt.float32, tag="ps", name=f"lg{et}")
            nc.tensor.matmul(
                lps[:], w2t[:].bitcast(dt.bfloat16), hsb[:], start=True, stop=True
            )
            osb = tails.tile([1, 512], dt.float32, tag="osb")
            nc.scalar.activation(
                osb[:], lps[:], mybir.ActivationFunctionType.Sigmoid,
                bias=b2t[:, 0:1],
            )
            nc.sync.dma_start(t["OUT"][:, 512 * et:512 * (et + 1)], osb[:])

        for et in range(N_ET):
            supertile(et)


def build_program():
    nc = bacc.Bacc("TRN2", target_bir_lowering=False)
    dt = mybir.dt
    t = {
        "A1": nc.dram_tensor(
            "A1", [U_PAD, ROW_U16], dt.uint16, kind="ExternalInput"
        ).ap(),
        "IDX": nc.dram_tensor("IDX", [128, IDX_COLS], dt.int16, kind="ExternalInput").ap(),
        "ZD": nc.dram_tensor(
            "ZD", [128, C_DDI * 8 * 2 * 128], dt.uint8, kind="ExternalInput"
        ).ap(),
        "ZP": nc.dram_tensor(
            "ZP", [128, C_DP * 8 * 2 * 128], dt.uint8, kind="ExternalInput"
        ).ap(),
        "W1": nc.dram_tensor("W1", [64, 4 * HID], dt.uint16, kind="ExternalInput").ap(),
        "B1": nc.dram_tensor("B1", [HID, 1], dt.float32, kind="ExternalInput").ap(),
        "W2": nc.dram_tensor("W2", [HID, 1], dt.uint16, kind="ExternalInput").ap(),
        "B2": nc.dram_tensor("B2", [1, 1], dt.float32, kind="ExternalInput").ap(),
        "OUT": nc.dram_tensor("OUT", [1, E_LOC], dt.float32, kind="ExternalOutput").ap(),
    }
    with tile.TileContext(nc) as tc:
        build_body(tc, t)
    nc.compile()
    return nc


def host_prep(z_drug, z_protein, ddi_ei, dp_ei, pred_ei, W1, b1, W2, b2):
    """Build the 8 per-core input maps (all numpy, no device work)."""
    z_drug = np.asarray(z_drug, np.float32)
    z_protein = np.asarray(z_protein, np.float32)
    ddi_ei = np.asarray(ddi_ei, np.int64)
    dp_ei = np.asarray(dp_ei, np.int64)
    pred_ei = np.asarray(pred_ei, np.int64)

    A_ddi = np.zeros((N_D, N_D), dtype=np.uint8)
    A_ddi[ddi_ei[0], ddi_ei[1]] = 1
    A_ddi[ddi_ei[1], ddi_ei[0]] = 1
    A_dp = np.zeros((N_D, N_P), dtype=np.uint8)
    A_dp[dp_ei[0], dp_ei[1]] = 1
    ddi_bits = np.packbits(A_ddi, axis=1, bitorder="little")   # [N_D, 1024]
    dp_bits = np.packbits(A_dp, axis=1, bitorder="little")     # [N_D, 512]

    zb_full = np.zeros((N_D, 128), dtype=np.float32)
    zb_full[:, :D_DIM] = z_drug
    zb_bytes = zb_full.astype(BF16).view(np.uint8)             # [N_D, 256]

    ZD = _pack_z_doublerow(z_drug)
    ZP = _pack_z_doublerow(z_protein)
    W1blocks = np.asarray(W1, np.float32).reshape(4, 64, HID)
    W1blocks = np.concatenate(
        [W1blocks[0:2], W1blocks[2:4][:, PERM, :]], axis=0
    )  # cn blocks arrive with rows permuted by PERM
    W1p = np.ascontiguousarray(
        W1blocks.astype(BF16).view(np.uint16).transpose(1, 0, 2).reshape(64, 4 * HID)
    )
    B1 = np.asarray(b1, np.float32).reshape(HID, 1)
    W2p = np.asarray(W2, np.float32).reshape(HID, 1).astype(BF16).view(np.uint16)
    B2 = np.asarray(b2, np.float32).reshape(1, 1)

    in_maps = []
    for c in range(N_CORES):
        s = pred_ei[0, c * E_LOC:(c + 1) * E_LOC]
        d = pred_ei[1, c * E_LOC:(c + 1) * E_LOC]
        rows = np.unique(np.concatenate([s, d]))
        nu = rows.shape[0]
        assert nu <= U_PAD
        remap_s = np.searchsorted(rows, s).astype(np.int16)
        remap_d = np.searchsorted(rows, d).astype(np.int16)
        A1 = np.zeros((U_PAD, 2 * ROW_U16), dtype=np.uint8)
        A1[:nu, 0:1024] = ddi_bits[rows]
        A1[:nu, 1024:1280] = zb_bytes[rows]
        A1[:nu, 1280:1792] = dp_bits[rows]

        cols = []
        for g in range(N_ET):
            cols.append(
                _wrap_idxs(
                    np.concatenate(
                        [remap_s[512 * g:512 * (g + 1)],
                         remap_d[512 * g:512 * (g + 1)]]
                    )
                )
            )
        idx = np.concatenate(cols, axis=1)
        assert idx.shape == (128, IDX_COLS)

        in_maps.append(
            {
                "A1": A1.view(np.uint16),
                "IDX": idx,
                "ZD": ZD,
                "ZP": ZP,
                "W1": W1p,
                "B1": B1,
                "W2": W2p,
                "B2": B2,
            }
        )
    return in_maps


def kernel(z_drug, z_protein, ddi_ei, dp_ei, pred_ei, W1, b1, W2, b2, _profile=None):
    from concourse.bass_utils import run_bass_kernel_spmd

    in_maps = host_prep(z_drug, z_protein, ddi_ei, dp_ei, pred_ei, W1, b1, W2, b2)
    nc = build_program()
    res = run_bass_kernel_spmd(
        nc,
        in_maps,
        core_ids=list(range(N_CORES)),
        **({} if _profile is None else _profile),
    )
    if _profile is not None:
        kernel.last_results = res
    out = np.concatenate([r["OUT"].reshape(-1) for r in res.results])
    return out.astype(np.float32)
